# revision 16
# baseline (speedup 1.0000x reference)
import sys
sys.path.insert(0, '/opt/trn_rl_repo')
import numpy as np
import concourse.bass as bass
import concourse.bacc as bacc
import concourse.tile as tile
import concourse.mybir as mybir
from concourse.bass_utils import run_bass_kernel_spmd

C3_TABLE = [(0, 1, 2), (1, 2, 3), (2, 3, 4), (3, 4, 5), (0, 4, 5), (0, 1, 5),
            (0, 1, 2, 3), (1, 2, 3, 4), (2, 3, 4, 5), (0, 3, 4, 5), (0, 1, 4, 5),
            (0, 1, 2, 5), (0, 1, 3, 4), (1, 2, 4, 5), (0, 2, 3, 5),
            (0, 1, 2, 3, 4, 5)]
A = 1.7159
S = 2.0 / 3.0

B, C, H, W = 256, 6, 142, 142
KH = KW = 5
OC = 16
OH, OW = H - 4, W - 4          # 138
NCORES = 8
B_LOC = B // NCORES            # 32

T = 6                          # oh rows per block
HH = T + KH - 1                # 10
NS2 = 2                        # kw taps packed into K (s dim)
K = C * HH * NS2               # 120
M = OC * T                     # 96
NP = 3                         # matmul passes: kw pairs {0,1},{2,3},{4,-}
BPER = 2
NS = BPER * OW                 # 276
NBLK = OH // T                 # 23 exactly
NPAIR = B_LOC // BPER          # 16
PGP = 8
NPG = NPAIR // PGP             # 2
GI = PGP * BPER                # 16
XF = GI * W                    # 2272
SF = PGP * NS                  # 2208

_cache = {}


def _build():
    if 'nc' in _cache:
        return _cache['nc']
    f32 = mybir.dt.float32
    f32r = mybir.dt.float32r
    nc = bacc.Bacc("TRN2", target_bir_lowering=False, debug=False,
                   num_devices=NCORES)
    x_d = nc.dram_tensor("x", [NBLK, K, B_LOC, W], f32, kind="ExternalInput").ap()
    w_d = nc.dram_tensor("w", [K, NP, M], f32, kind="ExternalInput").ap()
    b_d = nc.dram_tensor("b", [M, 1], f32, kind="ExternalInput").ap()
    y_d = nc.dram_tensor("y", [NBLK, NPG, M, SF], f32, kind="ExternalOutput").ap()

    with tile.TileContext(nc) as tc:
        with tc.tile_pool(name="wpool", bufs=1) as wpool, \
             tc.tile_pool(name="xpool", bufs=8) as xpool, \
             tc.tile_pool(name="tpool", bufs=4) as tpool, \
             tc.tile_pool(name="spool", bufs=3) as spool, \
             tc.tile_pool(name="pspool", bufs=1, space="PSUM") as pspool:
            w_sb = wpool.tile([K, NP * M], f32r)
            nc.sync.dma_start(w_sb[:], w_d[:].bitcast(f32r).rearrange("k f m -> k (f m)"))
            b_sb = wpool.tile([M, 1], f32)
            nc.sync.dma_start(b_sb[:], b_d[:])

            for blk in range(NBLK):
                for pg in range(NPG):
                    xt = xpool.tile([K, XF], f32r)
                    src = x_d[blk, :, pg * GI:(pg + 1) * GI, :]
                    src = src.rearrange("k i w -> k (i w)")
                    ieng = nc.sync if (blk * NPG + pg) % 2 == 0 else nc.gpsimd
                    ieng.dma_start(xt[:], src.bitcast(f32r))

                    stage = spool.tile([M, SF], f32)
                    xv = xt[:].rearrange("k (i w) -> k i w", i=GI)
                    pss = [pspool.tile([M, NS], f32, name=f"ps{p_}",
                                       tag=f"ps{p_}") for p_ in range(PGP)]
                    for g in range(NP):
                        for pair in range(PGP):
                            rv = xv[:, pair * BPER:(pair + 1) * BPER, :]
                            nc.tensor.matmul(
                                pss[pair][:],
                                w_sb[:, g * M:(g + 1) * M],
                                rv[:, :, 2 * g:2 * g + OW],
                                start=(g == 0), stop=(g == NP - 1),
                            )
                    for pair in range(PGP):
                        t_sb = tpool.tile([M, NS], f32)
                        nc.scalar.activation(t_sb[:], pss[pair][:],
                                             mybir.ActivationFunctionType.Tanh,
                                             bias=b_sb[:], scale=S)
                        nc.vector.tensor_scalar_mul(
                            stage[:, pair * NS:(pair + 1) * NS], t_sb[:], A)
                    oeng = nc.gpsimd if (blk * NPG + pg) % 2 == 0 else nc.sync
                    h = SF // 2
                    oeng.dma_start(y_d[blk, pg][:, 0:h], stage[:, 0:h])
                    oeng.dma_start(y_d[blk, pg][:, h:SF], stage[:, h:SF])
    nc.compile()
    _cache['nc'] = nc
    return nc


def _prep_weights(w3, b3, w4, b4, w6, b6):
    Wd = np.zeros((OC, C, KH, KW), np.float32)
    bias = np.zeros((OC,), np.float32)
    for i, idx in enumerate(C3_TABLE[:6]):
        Wd[i, list(idx)] = w3[i]
        bias[i] = b3[i]
    for i, idx in enumerate(C3_TABLE[6:15]):
        Wd[6 + i, list(idx)] = w4[i]
        bias[6 + i] = b4[i]
    Wd[15, list(C3_TABLE[15])] = w6[0]
    bias[15] = b6[0]

    # K row r = (c*HH + hh)*2 + s ; M col m = oc*T + j ; pass g: kw = 2g+s
    wk = np.zeros((K, NP, M), np.float32)
    for c in range(C):
        for hh in range(HH):
            for j in range(T):
                kh = hh - j
                if not (0 <= kh < KH):
                    continue
                for s in range(NS2):
                    for g in range(NP):
                        kw = 2 * g + s
                        if kw < KW:
                            r = (c * HH + hh) * 2 + s
                            wk[r, g, np.arange(OC) * T + j] = Wd[:, c, kh, kw]
    bvec = (S * bias[np.arange(M) // T]).reshape(M, 1).astype(np.float32)
    return wk, bvec


def _prep_x(x_shard):
    # [B_LOC, C, H, W] -> [NBLK, K=(c,hh,s), B_LOC, W]; s=1 shifted by one col
    xt = x_shard.transpose(1, 2, 0, 3)                  # [C, H, B, W]
    xb = np.zeros((NBLK, C, HH, NS2, B_LOC, W), np.float32)
    rows = (np.arange(NBLK) * T)[:, None] + np.arange(HH)[None, :]  # [23,10]
    g = xt[:, rows]                                     # [C, 23, 10, B, W]
    g = g.transpose(1, 0, 2, 3, 4)                      # [23, C, 10, B, W]
    xb[:, :, :, 0, :, :] = g
    xb[:, :, :, 1, :, :-1] = g[..., 1:]
    return np.ascontiguousarray(xb.reshape(NBLK, K, B_LOC, W))


def _unpack_y(y_s):
    v = y_s.reshape(NBLK, NPG, OC, T, PGP, BPER, OW)
    v = v.transpose(1, 4, 5, 2, 0, 3, 6)                # pg,pair,b2,oc,blk,j,ow
    return v.reshape(B_LOC, OC, OH, OW)


def kernel(x, w3, b3, w4, b4, w6, b6):
    nc = _build()
    w3, b3, w4, b4, w6, b6 = [np.asarray(a, dtype=np.float32)
                              for a in (w3, b3, w4, b4, w6, b6)]
    wk, bvec = _prep_weights(w3, b3, w4, b4, w6, b6)
    x = np.ascontiguousarray(np.asarray(x), dtype=np.float32)
    in_maps = [{"x": _prep_x(x[i * B_LOC:(i + 1) * B_LOC]), "w": wk, "b": bvec}
               for i in range(NCORES)]
    res = run_bass_kernel_spmd(nc, in_maps, list(range(NCORES)))
    out = np.concatenate([_unpack_y(res.results[i]["y"]) for i in range(NCORES)],
                         axis=0)
    return np.ascontiguousarray(out)


# revision 17
# speedup vs baseline: 1.0989x; 1.0989x over previous
import sys
sys.path.insert(0, '/opt/trn_rl_repo')
import numpy as np
import concourse.bass as bass
import concourse.bacc as bacc
import concourse.tile as tile
import concourse.mybir as mybir
from concourse.bass_utils import run_bass_kernel_spmd

C3_TABLE = [(0, 1, 2), (1, 2, 3), (2, 3, 4), (3, 4, 5), (0, 4, 5), (0, 1, 5),
            (0, 1, 2, 3), (1, 2, 3, 4), (2, 3, 4, 5), (0, 3, 4, 5), (0, 1, 4, 5),
            (0, 1, 2, 5), (0, 1, 3, 4), (1, 2, 4, 5), (0, 2, 3, 5),
            (0, 1, 2, 3, 4, 5)]
A = 1.7159
S = 2.0 / 3.0

B, C, H, W = 256, 6, 142, 142
KH = KW = 5
OC = 16
OH, OW = H - 4, W - 4          # 138
NCORES = 8
B_LOC = B // NCORES            # 32

T = 6                          # oh rows per block
HH = T + KH - 1                # 10
NS2 = 2                        # kw taps packed into K (s dim)
K = C * HH * NS2               # 120
M = OC * T                     # 96
NP = 3                         # matmul passes: kw pairs {0,1},{2,3},{4,-}
BPER = 2
NS = BPER * OW                 # 276
NBLK = OH // T                 # 23 exactly
NPAIR = B_LOC // BPER          # 16
PGP = 8
NPG = NPAIR // PGP             # 2
GI = PGP * BPER                # 16
XF = GI * W                    # 2272
SF = PGP * NS                  # 2208

_cache = {}


def _build():
    if 'nc' in _cache:
        return _cache['nc']
    f32 = mybir.dt.float32
    f32r = mybir.dt.float32r
    nc = bacc.Bacc("TRN2", target_bir_lowering=False, debug=False,
                   num_devices=NCORES)
    x_d = nc.dram_tensor("x", [NBLK, K, B_LOC, W], f32, kind="ExternalInput").ap()
    w_d = nc.dram_tensor("w", [K, NP, M], f32, kind="ExternalInput").ap()
    b_d = nc.dram_tensor("b", [M, 1], f32, kind="ExternalInput").ap()
    y_d = nc.dram_tensor("y", [NBLK, NPG, M, SF], f32, kind="ExternalOutput").ap()

    with tile.TileContext(nc) as tc:
        with tc.tile_pool(name="wpool", bufs=1) as wpool, \
             tc.tile_pool(name="xpool", bufs=8) as xpool, \
             tc.tile_pool(name="tpool", bufs=4) as tpool, \
             tc.tile_pool(name="spool", bufs=3) as spool, \
             tc.tile_pool(name="pspool", bufs=1, space="PSUM") as pspool:
            w_sb = wpool.tile([K, NP * M], f32r)
            nc.sync.dma_start(w_sb[:], w_d[:].bitcast(f32r).rearrange("k f m -> k (f m)"))
            b_sb = wpool.tile([M, 1], f32)
            nc.sync.dma_start(b_sb[:], b_d[:])

            for blk in range(NBLK):
                for pg in range(NPG):
                    xt = xpool.tile([K, XF], f32r)
                    src = x_d[blk, :, pg * GI:(pg + 1) * GI, :]
                    src = src.rearrange("k i w -> k (i w)")
                    nc.sync.dma_start(xt[:], src.bitcast(f32r))

                    stage = spool.tile([M, SF], f32)
                    xv = xt[:].rearrange("k (i w) -> k i w", i=GI)
                    pss = [pspool.tile([M, NS], f32, name=f"ps{p_}",
                                       tag=f"ps{p_}") for p_ in range(PGP)]
                    for g in range(NP):
                        for pair in range(PGP):
                            rv = xv[:, pair * BPER:(pair + 1) * BPER, :]
                            nc.tensor.matmul(
                                pss[pair][:],
                                w_sb[:, g * M:(g + 1) * M],
                                rv[:, :, 2 * g:2 * g + OW],
                                start=(g == 0), stop=(g == NP - 1),
                            )
                    for pair in range(PGP):
                        t_sb = tpool.tile([M, NS], f32)
                        nc.scalar.activation(t_sb[:], pss[pair][:],
                                             mybir.ActivationFunctionType.Tanh,
                                             bias=b_sb[:], scale=S)
                        nc.vector.tensor_scalar_mul(
                            stage[:, pair * NS:(pair + 1) * NS], t_sb[:], A)
                    nc.gpsimd.dma_start(y_d[blk, pg], stage[:])
    nc.compile()
    _cache['nc'] = nc
    return nc


def _prep_weights(w3, b3, w4, b4, w6, b6):
    Wd = np.zeros((OC, C, KH, KW), np.float32)
    bias = np.zeros((OC,), np.float32)
    for i, idx in enumerate(C3_TABLE[:6]):
        Wd[i, list(idx)] = w3[i]
        bias[i] = b3[i]
    for i, idx in enumerate(C3_TABLE[6:15]):
        Wd[6 + i, list(idx)] = w4[i]
        bias[6 + i] = b4[i]
    Wd[15, list(C3_TABLE[15])] = w6[0]
    bias[15] = b6[0]

    # K row r = (c*HH + hh)*2 + s ; M col m = oc*T + j ; pass g: kw = 2g+s
    wk = np.zeros((K, NP, M), np.float32)
    for c in range(C):
        for hh in range(HH):
            for j in range(T):
                kh = hh - j
                if not (0 <= kh < KH):
                    continue
                for s in range(NS2):
                    for g in range(NP):
                        kw = 2 * g + s
                        if kw < KW:
                            r = (c * HH + hh) * 2 + s
                            wk[r, g, np.arange(OC) * T + j] = Wd[:, c, kh, kw]
    bvec = (S * bias[np.arange(M) // T]).reshape(M, 1).astype(np.float32)
    return wk, bvec


def _prep_x(x_shard):
    # [B_LOC, C, H, W] -> [NBLK, K=(c,hh,s), B_LOC, W]; s=1 shifted by one col
    xt = x_shard.transpose(1, 2, 0, 3)                  # [C, H, B, W]
    xb = np.zeros((NBLK, C, HH, NS2, B_LOC, W), np.float32)
    rows = (np.arange(NBLK) * T)[:, None] + np.arange(HH)[None, :]  # [23,10]
    g = xt[:, rows]                                     # [C, 23, 10, B, W]
    g = g.transpose(1, 0, 2, 3, 4)                      # [23, C, 10, B, W]
    xb[:, :, :, 0, :, :] = g
    xb[:, :, :, 1, :, :-1] = g[..., 1:]
    return np.ascontiguousarray(xb.reshape(NBLK, K, B_LOC, W))


def _unpack_y(y_s):
    v = y_s.reshape(NBLK, NPG, OC, T, PGP, BPER, OW)
    v = v.transpose(1, 4, 5, 2, 0, 3, 6)                # pg,pair,b2,oc,blk,j,ow
    return v.reshape(B_LOC, OC, OH, OW)


def kernel(x, w3, b3, w4, b4, w6, b6):
    nc = _build()
    w3, b3, w4, b4, w6, b6 = [np.asarray(a, dtype=np.float32)
                              for a in (w3, b3, w4, b4, w6, b6)]
    wk, bvec = _prep_weights(w3, b3, w4, b4, w6, b6)
    x = np.ascontiguousarray(np.asarray(x), dtype=np.float32)
    in_maps = [{"x": _prep_x(x[i * B_LOC:(i + 1) * B_LOC]), "w": wk, "b": bvec}
               for i in range(NCORES)]
    res = run_bass_kernel_spmd(nc, in_maps, list(range(NCORES)))
    out = np.concatenate([_unpack_y(res.results[i]["y"]) for i in range(NCORES)],
                         axis=0)
    return np.ascontiguousarray(out)
